# revision 31
# baseline (speedup 1.0000x reference)
"""Graph-Transformer message-passing kernel for 8 Trainium2 NeuronCores.

Strategy (1D dst-shard edge parallelism, v2):
  - Nodes split into 8 contiguous ranges; core c owns all edges whose dst is
    in its range, so segment softmax/aggregation are fully local.
  - Per layer each core projects q|v for its own nodes (PE) into a bf16
    qv table slice, which is AllGathered into a global [8R, 256] bf16 table.
  - Host sorts each core's dst nodes by (hi-half in-degree, lo-half
    in-degree) into 128-node tiles; each tile has per-tile slot counts
    (wlo, whi) for edges sourced from the lo node half (global rows < 4R)
    and hi half.  Per-edge q|v rows are fetched with batched dma_gather
    (int16 indices; the 4R-row half-tables keep indices < 32768; pad slots
    point at row 0 and are masked by a -1e30 bias).
  - Edge softmax + weighted aggregation run on DVE/ACT in bf16; the output
    projection of layer l is algebraically fused with the QKV projection of
    layer l+1 (wo @ wqkv), so each tile does one transpose + one matmul.
  - v columns are stored d-major (head-dim major) so the probs broadcast
    in the pv multiply has a contiguous last dim (2x DVE mode).
  - 3 layers in one NEFF; host inverts the node permutation at the end.
"""

import numpy as np
import ml_dtypes

import concourse.bass as bass
import concourse.bacc as bacc
import concourse.mybir as mybir
import concourse.tile as tile
from concourse.masks import make_identity
from concourse.bass_utils import run_bass_kernel_spmd

NCORES = 8
L = 3
H = 8
D = 128
HD = D // H
NCLASS = 64
SCALE = 1.0 / float(np.sqrt(HD))
NEG = -1.0e30
P = 128
QV = 2 * D              # q|v row width (values)
GROUP_W = 80            # max total slot-columns buffered per group
SLOT_CHUNK = 8          # max slot-columns per dma_gather (1024 idxs; HW limit)

FP = mybir.dt.float32
BF = mybir.dt.bfloat16
I16 = mybir.dt.int16
AX = mybir.AxisListType
OP = mybir.AluOpType

BF_NP = ml_dtypes.bfloat16


# ----------------------------------------------------------------------------
# Host-side layout
# ----------------------------------------------------------------------------

class Layout:
    pass


def build_layout(src, dst, n_nodes):
    src = np.asarray(src).astype(np.int64)
    dst = np.asarray(dst).astype(np.int64)
    N = n_nodes
    chunk = (N + NCORES - 1) // NCORES
    half = (NCORES // 2) * chunk  # node id < half -> lo table

    deg = np.bincount(dst, minlength=N)
    lo_edge = src < half
    deg_lo = np.bincount(dst[lo_edge], minlength=N)
    deg_hi = deg - deg_lo

    # CSR of edges per dst, lo sources first
    order = np.lexsort((lo_edge == False, dst))  # noqa: E712  (dst asc, lo first)
    src_sorted = src[order]
    starts = np.zeros(N + 1, dtype=np.int64)
    np.cumsum(deg, out=starts[1:])

    # per-core work nodes sorted by (hi desc, lo desc); deg0 nodes to tail
    per_core_nodes = []
    n_work_max = 0
    for c in range(NCORES):
        nlo, nhi = c * chunk, min(N, (c + 1) * chunk)
        nodes = np.arange(nlo, nhi)
        work = nodes[deg[nodes] > 0]
        key = np.lexsort((-deg_lo[work], -deg_hi[work]))
        work = work[key]
        zero = nodes[deg[nodes] == 0]
        per_core_nodes.append((work, zero))
        n_work_max = max(n_work_max, len(work))

    TW = (n_work_max + P - 1) // P          # work tiles
    T = TW + 1                              # + tail tile (deg0 + pads)
    R = T * P

    # per-tile slot widths, shared across cores
    wlo = np.zeros(TW, dtype=np.int64)
    whi = np.zeros(TW, dtype=np.int64)
    for c in range(NCORES):
        work, _ = per_core_nodes[c]
        for t in range(TW):
            seg = work[t * P:(t + 1) * P]
            if len(seg):
                wlo[t] = max(wlo[t], deg_lo[seg].max())
                whi[t] = max(whi[t], deg_hi[seg].max())

    # perm + row_of
    perm = np.full((NCORES, R), -1, dtype=np.int64)
    row_of = np.full(N, -1, dtype=np.int64)
    for c in range(NCORES):
        work, zero = per_core_nodes[c]
        perm[c, :len(work)] = work
        perm[c, TW * P:TW * P + len(zero)] = zero
        row_of[work] = c * R + np.arange(len(work))
        row_of[zero] = c * R + TW * P + np.arange(len(zero))

    lo_rows = (NCORES // 2) * R             # row < lo_rows -> lo table

    # groups of tiles for batched gathers
    groups = []  # list of (t0, t1)
    t0 = 0
    while t0 < TW:
        t1, acc = t0, 0
        while t1 < TW and (t1 == t0 or acc + wlo[t1] + whi[t1] <= GROUP_W):
            acc += wlo[t1] + whi[t1]
            t1 += 1
        groups.append((t0, t1))
        t0 = t1

    # idx tables (int16, 16-wrapped, replicated to 128 partitions) and
    # per-(node,slot,head) bias tables, per core
    wsum = int((wlo + whi).sum())
    idx_cols = wsum * P // 16               # int16 cols of all groups concat
    bias_cols = wsum * H
    idx_tab = np.zeros((NCORES, 128, idx_cols), dtype=np.int16)
    bias_tab = np.full((NCORES, 128, bias_cols), NEG, dtype=np.float32)

    g_off = []                              # (lo_col_off, hi_col_off) per group
    b_off = np.zeros(TW + 1, dtype=np.int64)
    for t in range(TW):
        b_off[t + 1] = b_off[t] + (wlo[t] + whi[t]) * H

    col = 0
    for (t0, t1) in groups:
        klo = int(wlo[t0:t1].sum()) * P
        khi = int(whi[t0:t1].sum()) * P
        g_off.append((col, col + klo // 16))
        col += (klo + khi) // 16
    assert col == idx_cols

    for c in range(NCORES):
        work, _ = per_core_nodes[c]
        for gi, (t0, t1) in enumerate(groups):
            flat_lo = np.zeros((int(wlo[t0:t1].sum()), P), dtype=np.int16)
            flat_hi = np.zeros((int(whi[t0:t1].sum()), P), dtype=np.int16)
            slo = shi = 0
            for t in range(t0, t1):
                seg = work[t * P:(t + 1) * P]
                for p, node in enumerate(seg):
                    dl, dh = int(deg_lo[node]), int(deg_hi[node])
                    s0 = starts[node]
                    if dl:
                        flat_lo[slo:slo + dl, p] = row_of[
                            src_sorted[s0:s0 + dl]].astype(np.int16)
                        bias_tab[c, p, b_off[t]:b_off[t] + dl * H] = 0.0
                    if dh:
                        flat_hi[shi:shi + dh, p] = (
                            row_of[src_sorted[s0 + dl:s0 + dl + dh]] - lo_rows
                        ).astype(np.int16)
                        bias_tab[c, p,
                                 b_off[t] + wlo[t] * H:
                                 b_off[t] + wlo[t] * H + dh * H] = 0.0
                slo += int(wlo[t])
                shi += int(whi[t])
            lo0, hi0 = g_off[gi]
            for flat, off in ((flat_lo, lo0), (flat_hi, hi0)):
                k = flat.size
                if k == 0:
                    continue
                wrapped = flat.reshape(-1)  # j = s*128 + p order
                wrapped = wrapped.reshape(k // 16, 16).T  # [16, k/16]
                idx_tab[c, :, off:off + k // 16] = np.tile(wrapped, (8, 1))

    lay = Layout()
    lay.N, lay.R, lay.T, lay.TW, lay.chunk, lay.half = N, R, T, TW, chunk, half
    lay.lo_rows = lo_rows
    lay.wlo, lay.whi = wlo, whi
    lay.groups, lay.g_off, lay.b_off = groups, g_off, b_off
    lay.idx_cols, lay.bias_cols, lay.wsum = idx_cols, bias_cols, wsum
    lay.perm, lay.row_of = perm, row_of
    lay.idx_tab, lay.bias_tab = idx_tab, bias_tab
    return lay


def _vperm():
    """v column permutation: new col d*H+h = old col h*HD+d."""
    j = np.arange(D)
    d, h = j // H, j % H
    return h * HD + d


def host_weights(qkv_w, qkv_b, out_w, out_b, out_w_last, out_b_last):
    """Build fused per-layer weights (fp32 math, cast to bf16 at the end)."""
    vp = _vperm()
    wc, bc = [], []  # full qkv projection [D, 3D], [3D] per layer
    for l in range(L):
        wq, wk, wv = qkv_w[l, 0], qkv_w[l, 1], qkv_w[l, 2]
        bq, bk, bv = qkv_b[l, 0], qkv_b[l, 1], qkv_b[l, 2]
        wc.append(np.concatenate([wq, wk * SCALE, wv[:, vp]], axis=1))
        bc.append(np.concatenate([bq, bk * SCALE, bv[vp]]))
    # layer0 projection from x
    W0, B0 = wc[0], bc[0]
    # fused: agg' (d-major) @ (wo_perm @ wc[l+1]) + (bo @ wc[l+1] + bc[l+1])
    WFs, BFs, qvtail = [], [], []
    for l in range(L - 1):
        wo_p = out_w[l][vp, :]              # rows permuted to agg' order
        WFs.append(wo_p @ wc[l + 1])
        BFs.append(out_b[l] @ wc[l + 1] + bc[l + 1])
        # deg0/tail rows: x_{l+1} = bo[l]; their next-layer q|v row constant
        qkv_t = out_b[l] @ wc[l + 1] + bc[l + 1]
        qvtail.append(np.concatenate([qkv_t[0:D], qkv_t[2 * D:3 * D]]))
    WL = out_w_last[vp, :]
    BL = out_b_last
    return dict(W0=W0, B0=B0, WFs=WFs, BFs=BFs, qvtail=qvtail, WL=WL, BL=BL)


def host_inputs(lay, x, wts):
    x = np.asarray(x, dtype=np.float32)
    W0 = wts["W0"].astype(BF_NP)
    B0 = np.tile(wts["B0"][None, :], (P, 1)).astype(BF_NP)
    WF = np.stack(wts["WFs"]).astype(BF_NP)                  # [2, D, 3D]
    BFu = np.stack([np.tile(b[None, :], (P, 1)) for b in wts["BFs"]]).astype(
        BF_NP)                                               # [2, P, 3D]
    qvt = np.stack([np.tile(q[None, :], (P, 1)) for q in wts["qvtail"]]).astype(
        BF_NP)                                               # [2, P, 2D]
    WL = wts["WL"].astype(BF_NP)                             # [D, NCLASS]
    BL = np.tile(wts["BL"][None, :], (P, 1)).astype(BF_NP)

    in_maps = []
    for c in range(NCORES):
        sel = lay.perm[c]
        x0 = np.where((sel >= 0)[:, None], x[np.maximum(sel, 0)], 0.0)
        m = {
            "x0": x0.astype(BF_NP),
            "W0": W0, "B0": B0, "WF": WF, "BFu": BFu, "qvt": qvt,
            "WL": WL, "BL": BL,
            "idx": lay.idx_tab[c],
            "bias": lay.bias_tab[c].astype(BF_NP),
        }
        in_maps.append(m)
    return in_maps


def host_output(lay, outs):
    full = np.zeros((lay.N, NCLASS), dtype=np.float32)
    for c in range(NCORES):
        real = lay.perm[c] >= 0
        full[lay.perm[c][real]] = outs[c][real]
    return full


# ----------------------------------------------------------------------------
# Device program
# ----------------------------------------------------------------------------

def build_nc(lay, iters=1):
    R, T, TW = lay.R, lay.T, lay.TW
    wlo, whi, groups = lay.wlo, lay.whi, lay.groups
    nc = bacc.Bacc(trn_type="TRN2", num_devices=NCORES)

    x0 = nc.dram_tensor("x0", [R, D], BF, kind="ExternalInput")
    W0_d = nc.dram_tensor("W0", [D, 3 * D], BF, kind="ExternalInput")
    B0_d = nc.dram_tensor("B0", [P, 3 * D], BF, kind="ExternalInput")
    WF_d = nc.dram_tensor("WF", [L - 1, D, 3 * D], BF, kind="ExternalInput")
    BF_d = nc.dram_tensor("BFu", [L - 1, P, 3 * D], BF, kind="ExternalInput")
    qvt_d = nc.dram_tensor("qvt", [L - 1, P, QV], BF, kind="ExternalInput")
    WL_d = nc.dram_tensor("WL", [D, NCLASS], BF, kind="ExternalInput")
    BL_d = nc.dram_tensor("BL", [P, NCLASS], BF, kind="ExternalInput")
    idx_d = nc.dram_tensor("idx", [128, lay.idx_cols], I16, kind="ExternalInput")
    bias_d = nc.dram_tensor("bias", [128, lay.bias_cols], BF, kind="ExternalInput")
    out_ext = nc.dram_tensor("out", [R, NCLASS], FP, kind="ExternalOutput")

    qv_slice = nc.dram_tensor("qv_slice", [R, QV], BF, kind="Internal")
    qv_full = nc.dram_tensor(
        "qv_full", [NCORES * R, QV], BF, kind="Internal", addr_space="Shared"
    )
    rg = [list(range(NCORES))]
    lo_rows = lay.lo_rows

    with tile.TileContext(nc) as tc:
        with (
            nc.allow_low_precision(reason="bf16 edge softmax; 2e-2 tolerance"),
            tc.tile_pool(name="const", bufs=1) as cpool,
            tc.tile_pool(name="persist", bufs=1) as ppool,
            tc.tile_pool(name="gbuf", bufs=2) as gpool,
            tc.tile_pool(name="tmp", bufs=2) as tpool,
            tc.tile_pool(name="pv", bufs=2) as vpool,
            tc.tile_pool(name="proj", bufs=3) as projpool,
            tc.tile_pool(name="small", bufs=3) as spool,
            tc.tile_pool(name="psum_t", bufs=2, space="PSUM") as ps_t,
            tc.tile_pool(name="psum_q", bufs=2, space="PSUM") as ps_q,
        ):
            ident = cpool.tile([P, P], BF, tag="ident", name="ident")
            make_identity(nc, ident[:])
            e0 = cpool.tile([P, P], BF, tag="e0", name="e0")
            nc.vector.memset(e0[:], 0.0)
            nc.vector.memset(e0[0:1, :], 1.0)
            W0_sb = cpool.tile([P, 3 * D], BF, tag="W0", name="W0")
            nc.sync.dma_start(W0_sb[:], W0_d[:])
            B0_sb = cpool.tile([P, 3 * D], BF, tag="B0", name="B0")
            nc.sync.dma_start(B0_sb[:], B0_d[:])
            WF_sb = cpool.tile([P, (L - 1) * 3 * D], BF, tag="WF", name="WF")
            nc.sync.dma_start(
                WF_sb[:].rearrange("k (l n) -> k l n", l=L - 1),
                WF_d[:].rearrange("l k n -> k l n"),
            )
            BF_sb = cpool.tile([P, (L - 1) * 3 * D], BF, tag="BFu", name="BFu")
            nc.sync.dma_start(
                BF_sb[:].rearrange("p (l n) -> p l n", l=L - 1),
                BF_d[:].rearrange("l p n -> p l n"),
            )
            qvt_sb = cpool.tile([P, (L - 1) * QV], BF, tag="qvt", name="qvt")
            nc.sync.dma_start(
                qvt_sb[:].rearrange("p (l n) -> p l n", l=L - 1),
                qvt_d[:].rearrange("l p n -> p l n"),
            )
            WL_sb = cpool.tile([P, NCLASS], BF, tag="WL", name="WL")
            nc.sync.dma_start(WL_sb[:], WL_d[:])
            BL_sb = cpool.tile([P, NCLASS], BF, tag="BL", name="BL")
            nc.sync.dma_start(BL_sb[:], BL_d[:])
            idx_sb = cpool.tile([128, lay.idx_cols], I16, tag="idx", name="idx")
            nc.sync.dma_start(idx_sb[:], idx_d[:])
            bias_sb = cpool.tile([128, lay.bias_cols], BF, tag="bias", name="bias")
            nc.sync.dma_start(bias_sb[:], bias_d[:])

            k_sb = ppool.tile([P, TW * D], BF, tag="k", name="k")
            x_sb = ppool.tile([P, T * D], BF, tag="x", name="x")
            nc.sync.dma_start(
                x_sb[:].rearrange("p (t f) -> p t f", f=D),
                x0[:].rearrange("(t p) f -> p t f", p=P),
            )
            tc.strict_bb_all_engine_barrier()

            def project(lhsT_src_ap, t, l):
                """transpose + fused qkv matmul for tile t feeding layer l+1."""
                xT_ps = ps_t.tile([P, P], BF, tag="xT", name="xT")
                nc.tensor.transpose(xT_ps[:], lhsT_src_ap, ident[:])
                xT = projpool.tile([P, P], BF, tag="xT_sb", name="xT_sb")
                nc.scalar.copy(xT[:], xT_ps[:])
                qkv_ps = ps_q.tile([P, 3 * D], FP, tag="qkv", name="qkv")
                if l < 0:
                    w_ap = W0_sb[:]
                    b_ap = B0_sb[:]
                else:
                    w_ap = WF_sb[:, l * 3 * D:(l + 1) * 3 * D]
                    b_ap = BF_sb[:, l * 3 * D:(l + 1) * 3 * D]
                # bias preload: e0^T @ bias_rows broadcasts partition-0 bias
                # over all nodes; the projection accumulates on top in PSUM.
                nc.tensor.matmul(qkv_ps[:], lhsT=e0[:], rhs=b_ap,
                                 start=True, stop=False)
                nc.tensor.matmul(qkv_ps[:], lhsT=xT[:], rhs=w_ap,
                                 start=False, stop=True)
                qv_st = projpool.tile([P, QV], BF, tag="qv_st", name="qv_st")
                nc.scalar.copy(
                    qv_st[:].rearrange("p (b f) -> p b f", f=D),
                    qkv_ps[:].rearrange("p (b f) -> p b f", f=D)[:, 0::2, :])
                if t < TW:
                    nc.scalar.copy(
                        k_sb[:, t * D:(t + 1) * D], qkv_ps[:, D:2 * D])
                nc.sync.dma_start(qv_slice[t * P:(t + 1) * P, :], qv_st[:])

            # ---- layer 0 projection (from x) ----
            for rep in range(iters):
              for t in range(T):
                project(x_sb[:, t * D:(t + 1) * D], t, -1)

              for l in range(L):
                nc.gpsimd.collective_compute(
                    "AllGather", OP.bypass, replica_groups=rg,
                    ins=[qv_slice[:, :]], outs=[qv_full[:, :]],
                )
                tc.strict_bb_all_engine_barrier()

                def issue_gathers(gi):
                    t0, t1 = groups[gi]
                    klo = int(wlo[t0:t1].sum())
                    khi = int(whi[t0:t1].sum())
                    gb = gpool.tile([P, (klo + khi) * QV], BF, tag="gb",
                                    name=f"gb{gi}")
                    lo_c, hi_c = lay.g_off[gi]
                    # HW dma_gather is stable at <=1024 idxs (8 slot-cols);
                    # larger num_idxs crashes the exec unit.
                    for col0, ncols, view, icol in (
                        (0, klo, qv_full[0:lo_rows, :], lo_c),
                        (klo, khi, qv_full[lo_rows:, :], hi_c),
                    ):
                        for s0 in range(0, ncols, SLOT_CHUNK):
                            sc = min(SLOT_CHUNK, ncols - s0)
                            nc.gpsimd.dma_gather(
                                out_ap=gb[:, (col0 + s0) * QV:
                                          (col0 + s0 + sc) * QV].rearrange(
                                    "p (s e) -> p s e", e=QV),
                                in_ap=view,
                                idxs_ap=idx_sb[:, icol + s0 * P // 16:
                                               icol + (s0 + sc) * P // 16],
                                num_idxs=sc * P, num_idxs_reg=sc * P,
                                elem_size=QV,
                            )
                    return gb, klo

                pending = issue_gathers(0)
                for gi, (t0, t1) in enumerate(groups):
                    gb, klo = pending
                    if gi + 1 < len(groups):
                        pending = issue_gathers(gi + 1)
                    gw = int((wlo[t0:t1] + whi[t0:t1]).sum())
                    gboff = int(lay.b_off[t0])
                    tmp_g = tpool.tile([P, gw * D], BF, tag="tmp", name="tmp")
                    s_g = spool.tile([P, gw * H], BF, tag="s", name="s")
                    ex_g = spool.tile([P, gw * H], BF, tag="ex", name="ex")
                    slo = shi = toff = 0
                    for t in range(t0, t1):
                        a, b = int(wlo[t]), int(whi[t])
                        w = a + b
                        if w == 0:
                            continue
                        q_lo = gb[:].rearrange("p (s e) -> p s e", e=QV)[
                            :, slo:slo + a, 0:D]
                        q_hi = gb[:].rearrange("p (s e) -> p s e", e=QV)[
                            :, klo + shi:klo + shi + b, 0:D]
                        k_ap1 = k_sb[:, t * D:(t + 1) * D]
                        if a:
                            nc.vector.tensor_tensor(
                                out=tmp_g[:, toff * D:(toff + a) * D].rearrange(
                                    "p (s f) -> p s f", f=D),
                                in0=q_lo,
                                in1=k_ap1.unsqueeze(1).to_broadcast([P, a, D]),
                                op=OP.mult)
                        if b:
                            nc.vector.tensor_tensor(
                                out=tmp_g[:, (toff + a) * D:
                                          (toff + w) * D].rearrange(
                                    "p (s f) -> p s f", f=D),
                                in0=q_hi,
                                in1=k_ap1.unsqueeze(1).to_broadcast([P, b, D]),
                                op=OP.mult)
                        slo += a
                        shi += b
                        toff += w
                    # group-merged d-fold add tree (2x mode; TensorReduce is
                    # 1x), bias add and exp: all elementwise over the packed
                    # per-tile segments, so one instruction per group.
                    dw = HD // 2
                    while dw >= 1:
                        nc.vector.tensor_tensor(
                            out=(s_g[:].unsqueeze(2) if dw == 1 else
                                 tmp_g[:].rearrange(
                                     "p (s d) -> p s d", d=HD)[:, :, 0:dw]),
                            in0=tmp_g[:].rearrange(
                                "p (s d) -> p s d", d=HD)[:, :, 0:dw],
                            in1=tmp_g[:].rearrange(
                                "p (s d) -> p s d", d=HD)[:, :, dw:2 * dw],
                            op=OP.add)
                        dw //= 2
                    nc.vector.tensor_tensor(
                        out=s_g[:], in0=s_g[:],
                        in1=bias_sb[:, gboff:gboff + gw * H], op=OP.add)
                    # scores are O(1)-bounded: skip the max-subtract
                    # (exp cannot overflow; all-pad rows are discarded)
                    nc.scalar.activation(
                        out=ex_g[:], in_=s_g[:],
                        func=mybir.ActivationFunctionType.Exp)
                    slo = shi = toff = 0
                    for t in range(t0, t1):
                        a, b = int(wlo[t]), int(whi[t])
                        w = a + b
                        if w == 0:
                            continue
                        v_lo = gb[:].rearrange("p (s e) -> p s e", e=QV)[
                            :, slo:slo + a, D:QV]
                        v_hi = gb[:].rearrange("p (s e) -> p s e", e=QV)[
                            :, klo + shi:klo + shi + b, D:QV]
                        ex = ex_g[:, toff * H:(toff + w) * H]
                        denom = spool.tile([P, H], FP, tag="denom", name="denom")
                        nc.vector.reduce_sum(
                            denom[:],
                            ex.rearrange("p (s h) -> p h s", h=H),
                            axis=AX.X)
                        rec_bf = spool.tile([P, H], BF, tag="recb", name="recb")
                        nc.vector.reciprocal(rec_bf[:], denom[:])
                        # pv = v * ex (unnormalized), both halves into one buf
                        pv = vpool.tile([P, w * D], BF, tag="pv", name="pv")
                        if a:
                            nc.vector.tensor_tensor(
                                out=pv[:, 0:a * D].rearrange(
                                    "p (s d h) -> p s d h", d=HD, h=H),
                                in0=v_lo.rearrange("p s (d h) -> p s d h", h=H),
                                in1=ex[:, 0:a * H].rearrange(
                                    "p (s h) -> p s h", h=H
                                ).unsqueeze(2).to_broadcast([P, a, HD, H]),
                                op=OP.mult)
                        if b:
                            nc.vector.tensor_tensor(
                                out=pv[:, a * D:].rearrange(
                                    "p (s d h) -> p s d h", d=HD, h=H),
                                in0=v_hi.rearrange("p s (d h) -> p s d h", h=H),
                                in1=ex[:, a * H:].rearrange(
                                    "p (s h) -> p s h", h=H
                                ).unsqueeze(2).to_broadcast([P, b, HD, H]),
                                op=OP.mult)
                        # tree-reduce over slots
                        width = w
                        while width > 1:
                            hw = (width + 1) // 2
                            nc.vector.tensor_tensor(
                                out=pv[:, 0:(width - hw) * D],
                                in0=pv[:, 0:(width - hw) * D],
                                in1=pv[:, hw * D:width * D],
                                op=OP.add)
                            width = hw
                        # normalize: agg = pv[0] * rec (broadcast over d)
                        agg = projpool.tile([P, D], BF, tag="agg", name="agg")
                        nc.vector.tensor_tensor(
                            out=agg[:].rearrange("p (d h) -> p d h", h=H),
                            in0=pv[:, 0:D].rearrange("p (d h) -> p d h", h=H),
                            in1=rec_bf[:].unsqueeze(1).to_broadcast([P, HD, H]),
                            op=OP.mult)
                        if l < L - 1:
                            project(agg[:], t, l)
                        else:
                            aT_ps = ps_t.tile([P, P], BF, tag="xT", name="aT")
                            nc.tensor.transpose(aT_ps[:], agg[:], ident[:])
                            aT = projpool.tile([P, P], BF, tag="xT_sb",
                                               name="aT_sb")
                            nc.scalar.copy(aT[:], aT_ps[:])
                            o_ps = ps_q.tile([P, NCLASS], FP, tag="o", name="o")
                            nc.tensor.matmul(o_ps[:], lhsT=e0[:],
                                             rhs=BL_sb[:],
                                             start=True, stop=False)
                            nc.tensor.matmul(o_ps[:], lhsT=aT[:], rhs=WL_sb[:],
                                             start=False, stop=True)
                            o_sb = projpool.tile([P, NCLASS], FP, tag="o_sb",
                                                 name="o_sb")
                            nc.scalar.copy(o_sb[:], o_ps[:])
                            nc.sync.dma_start(
                                out_ext[t * P:(t + 1) * P, :], o_sb[:])
                        slo += a
                        shi += b
                        toff += w
                # tail tile (deg0 + pad rows)
                if l < L - 1:
                    # next-layer q|v rows of tail nodes are a host constant
                    nc.sync.dma_start(
                        qv_slice[TW * P:(TW + 1) * P, :],
                        qvt_sb[:, l * QV:(l + 1) * QV])
                else:
                    o_sb = projpool.tile([P, NCLASS], FP, tag="o_sb",
                                         name="o_tail")
                    nc.vector.tensor_copy(o_sb[:], BL_sb[:])
                    nc.sync.dma_start(
                        out_ext[TW * P:(TW + 1) * P, :], o_sb[:])
    nc.compile()
    return nc


# ----------------------------------------------------------------------------
# Entry point
# ----------------------------------------------------------------------------

_trace = [False]  # test.py can flip this to profile


def kernel(x, src, dst, qkv_w, qkv_b, out_w, out_b, out_w_last, out_b_last):
    x = np.asarray(x, dtype=np.float32)
    lay = build_layout(np.asarray(src), np.asarray(dst), x.shape[0])
    wts = host_weights(
        np.asarray(qkv_w, dtype=np.float32), np.asarray(qkv_b, dtype=np.float32),
        np.asarray(out_w, dtype=np.float32), np.asarray(out_b, dtype=np.float32),
        np.asarray(out_w_last, dtype=np.float32),
        np.asarray(out_b_last, dtype=np.float32),
    )
    in_maps = host_inputs(lay, x, wts)
    nc = build_nc(lay)
    kernel.last_nc = nc
    res = run_bass_kernel_spmd(
        nc, in_maps, core_ids=list(range(NCORES)), trace=_trace[0]
    )
    kernel.last_results = res
    outs = [res.results[c]["out"] for c in range(NCORES)]
    return host_output(lay, outs)
